# revision 23
# baseline (speedup 1.0000x reference)
"""Trainium2 Bass kernel for nn_AssociatorLoss.

Reference (B=32, N=32), a = cayley_cube (B,N,N,N):
    one[b,i,j,k,l] = sum_m a[b,i,m,l] * a[b,j,k,m]
    two[b,i,j,k,l] = sum_m a[b,m,k,l] * a[b,i,j,m]
    kl = sum(two * (log(two) - log(one))) / B

Data-parallel over b (4 per core, 8 cores, host combines partial sums).

Per batch element b, chunk c covers i in [4c,4c+4).  The two einsums are
K=32 matmuls; with PE row-tiling (tile_position=(32r,0), 32x128 mode)
the 4 matmuls of a chunk (one/two x column-half) occupy the four row
bands of the PE array and run concurrently.  Operands are host-stacked
into quad tensors: stq = [st1;st1;st2;st2], mvq = [mv1;mv1;mv2;mv2]
(all bf16), so band r reads SBUF partitions 32r..32r+32.

Column order of both products is paired: f = kH*64 + x*2 + kL (x = l
for two / j for one, k = 2kH+kL), chosen so that viewing a bf16 tensor
as uint32 pairs and doing a 32x32-block StreamTranspose on DVE performs
exactly the one-layout -> two-layout alignment at half cost.

Per chunk (log-difference pipeline):
    lt  = Ln(1024*tp)   ACT  (evacuates tp from PSUM, bf16)
    lo  = Ln(1024*op)   ACT  (evacuates op, one-layout)
    loT = pairT(lo)     DVE  (uint32 StreamTranspose -> two-layout)
    dd  = lt - loT      DVE or Pool (bf16; sum two*dd = sum two*log(two/one),
                         the Ln scale offsets cancel)

Dot products ride the PE in 128x128 mode, batched per group of 4
chunks (one tiling-mode switch each way per group, software-pipelined
one group behind the row-tiled matmuls): G[m,f] += sum_p t8c[p,m]*dd[p,f]
accumulated over the group's 4 chunks in PSUM, then one stt per half:
    acc[:, col] = sum(ab * G)        DVE
which evaluates sum(two * dd) exactly (t8c[p=(di,j),m]=a[i,j,m],
ab[m,f]=a[m,k,l] in bf16, f32 accumulation).

kl = sum(acc) / B on the host in float64.
"""

import sys

for _p in ("/opt/trn_rl_repo",):
    if _p not in sys.path:
        sys.path.insert(0, _p)

import ml_dtypes
import numpy as np

import concourse.bacc as bacc
import concourse.mybir as mybir
import concourse.tile as tile
from concourse.bass_utils import run_bass_kernel_spmd

B, N = 32, 32
N_CORES = 8
B_LOCAL = B // N_CORES      # 4
NCHUNK = 8                  # chunks of 128 (i,*) rows per batch element
NGROUP = B_LOCAL * NCHUNK // 4   # 8 groups of 4 chunks
F32 = mybir.dt.float32
BF16 = mybir.dt.bfloat16
LN_SCALE = 1024.0           # centers ln() inputs near 0 for bf16 precision

# fraction of dd-subtracts routed to Pool (SBUF-only engine)
DD_ON_POOL = frozenset({1, 3, 5, 7, 9, 11, 13, 15, 17, 19, 21, 23, 25, 27, 29, 31})


def _bf16(x):
    return np.ascontiguousarray(x).astype(ml_dtypes.bfloat16)


def host_prep(a_local: np.ndarray):
    """a_local [B_LOCAL, N, N, N] f32 -> operand dict (per-core inputs)."""
    out = {}
    for b in range(B_LOCAL):
        A = np.ascontiguousarray(a_local[b], dtype=np.float32)
        st2 = A.transpose(2, 0, 1).reshape(N, N * N)    # [m,(i,j)]
        st1 = A.transpose(1, 0, 2).reshape(N, N * N)    # [m,(i,l)]
        # paired column orders: col = kH*64 + x*2 + kL
        mv2p = A.reshape(N, 16, 2, N).transpose(0, 1, 3, 2).reshape(N, N * N)
        mv1p = (A.transpose(2, 1, 0).reshape(N, 16, 2, N)
                .transpose(0, 1, 3, 2).reshape(N, N * N))
        # t8[di*32+j, c*32+m] = a[4c+di, j, m]
        t8 = A.reshape(NCHUNK, 4, N, N).transpose(1, 2, 0, 3).reshape(128, 256)
        out[f"stq_{b}"] = _bf16(np.concatenate([st1, st1, st2, st2], axis=0))
        out[f"mvq_{b}"] = _bf16(np.concatenate([mv1p, mv1p, mv2p, mv2p],
                                               axis=0))
        out[f"t8_{b}"] = _bf16(t8)
        out[f"ab_{b}"] = _bf16(mv2p)
    return out


def build():
    nc = bacc.Bacc(None, target_bir_lowering=False)
    mult = mybir.AluOpType.mult
    subtract = mybir.AluOpType.subtract
    Ln = mybir.ActivationFunctionType.Ln

    ext = {}
    for b in range(B_LOCAL):
        for nm, shape in (("stq", [128, N * N]), ("mvq", [128, N * N]),
                          ("t8", [128, 256]), ("ab", [N, N * N])):
            ext[f"{nm}_{b}"] = nc.declare_dram_parameter(
                f"{nm}_{b}", shape, BF16, isOutput=False)
    out_ext = nc.declare_dram_parameter("out", [N, 2 * NGROUP], F32,
                                        isOutput=True)

    with tile.TileContext(nc) as tc:
        with (
            tc.tile_pool(name="apool", bufs=2) as apool,
            tc.tile_pool(name="spool", bufs=6) as spool,
            tc.tile_pool(name="scratch", bufs=1) as scratch,
            tc.tile_pool(name="accp", bufs=1) as accpool,
            tc.tile_pool(name="psumO", bufs=1, space="PSUM") as psumO,
            tc.tile_pool(name="psumT", bufs=2, space="PSUM") as psumT,
            tc.tile_pool(name="psumG", bufs=2, space="PSUM") as psumG,
        ):
            acc = accpool.tile([N, 2 * NGROUP], F32)
            p1 = scratch.tile([N, 512], BF16)
            p2 = scratch.tile([N, 512], BF16)

            # deferred G-dot work: (g4 tiles, t8 tile, dd tiles, ab, gid)

            pending = []

            def flush_pending():
                if not pending:
                    return
                g4, t8t, dds, abt, gid = pending.pop()
                for cc in range(4):
                    c = gid % 2 * 4 + cc
                    for h in range(2):
                        cs = slice(512 * h, 512 * (h + 1))
                        nc.tensor.matmul(
                            g4[32 * h:32 * (h + 1), :],
                            t8t[:, 32 * c:32 * (c + 1)],
                            dds[cc][:, cs], start=(cc == 0), stop=(cc == 3),
                            skip_group_check=True)
                for h in range(2):
                    col = gid * 2 + h
                    cs = slice(512 * h, 512 * (h + 1))
                    nc.vector.scalar_tensor_tensor(
                        out=p1[:], in0=g4[32 * h:32 * (h + 1), :], scalar=1.0,
                        in1=abt[:, cs], op0=mult, op1=mult,
                        accum_out=acc[:, col:col + 1])

            chunk_idx = 0
            for b in range(B_LOCAL):
                t = {}
                for nm, shape in (("stq", [128, N * N]), ("mvq", [128, N * N]),
                                  ("t8", [128, 256]), ("ab", [N, N * N])):
                    tt = apool.tile(shape, BF16, tag=nm, name=nm)
                    nc.sync.dma_start(out=tt[:], in_=ext[f"{nm}_{b}"][:])
                    t[nm] = tt

                for g in range(2):          # 2 groups of 4 chunks per b
                    gid = b * 2 + g
                    g4 = psumG.tile([64, 512], F32, tag="g4", name="g4")
                    dds = []
                    for cc in range(4):     # chunk within group
                        c = g * 4 + cc
                        ms = slice(128 * c, 128 * (c + 1))
                        op = psumO.tile([128, 1024], F32, tag="op", name="op")
                        tp = psumT.tile([128, 1024], F32, tag="tp", name="tp")
                        for h in range(2):
                            cs = slice(512 * h, 512 * (h + 1))
                            nc.tensor.matmul(op[:, cs], t["stq"][0:32, ms],
                                             t["mvq"][0:32, cs], start=True,
                                             stop=True)
                        for h in range(2):
                            cs = slice(512 * h, 512 * (h + 1))
                            nc.tensor.matmul(tp[:, cs], t["stq"][64:96, ms],
                                             t["mvq"][64:96, cs], start=True,
                                             stop=True)

                        lo = spool.tile([128, 1024], BF16, tag="lo")
                        nc.scalar.activation(lo[:], op[:], Ln, scale=LN_SCALE)
                        lt = spool.tile([128, 1024], BF16, tag="lt")
                        nc.scalar.activation(lt[:], tp[:], Ln, scale=LN_SCALE)
                        loT = spool.tile([128, 1024], BF16, tag="loT")
                        nc.vector.transpose(
                            loT[:].bitcast(mybir.dt.uint32),
                            lo[:].bitcast(mybir.dt.uint32))
                        dd = spool.tile([128, 1024], BF16, tag="dd", bufs=9)
                        if chunk_idx in DD_ON_POOL:
                            nc.gpsimd.tensor_tensor(out=dd[:], in0=lt[:],
                                                    in1=loT[:], op=subtract)
                        else:
                            nc.vector.tensor_tensor(out=dd[:], in0=lt[:],
                                                    in1=loT[:], op=subtract)
                        dds.append(dd)
                        chunk_idx += 1

                    flush_pending()
                    pending.append((g4, t["t8"], dds, t["ab"], gid))

            flush_pending()
            nc.sync.dma_start(out=out_ext[:, :], in_=acc[:])

    nc.compile()
    return nc


def kernel(cayley_cube: np.ndarray) -> np.ndarray:
    assert cayley_cube.shape == (B, N, N, N)
    nc = build()
    shards = cayley_cube.reshape(N_CORES, B_LOCAL, N, N, N)
    in_maps = [host_prep(shards[i]) for i in range(N_CORES)]
    res = run_bass_kernel_spmd(nc, in_maps, core_ids=list(range(N_CORES)))
    tot = np.float64(0.0)
    for r in res.results:
        tot += r["out"].sum(dtype=np.float64)
    return np.float32(tot / B)


if __name__ == "__main__":
    rng = np.random.default_rng(0)
    raw = rng.uniform(0.05, 1.0, size=(B, N, N, N)).astype(np.float32)
    a = raw / raw.sum(axis=-1, keepdims=True)
    print(kernel(a))
